# revision 1
# baseline (speedup 1.0000x reference)
"""Trainium2 Bass kernel for nn_AttentionLayer (cross-attention + FF + LayerNorm).

Strategy:
  - Data-parallel over batch: 16 batch elements -> 8 NeuronCores, 2 each.
  - Feature-major dataflow on chip: activations live as [feature, token] so
    every matmul contracts over the partition dim with zero transposes except
    the initial text/image load (PE transpose) and the final LN (PE transpose
    back to token-major).
  - All matmuls run as float32r (TF32-like; full PE rate at moving-dim >= 256)
    with fp32 PSUM accumulation.
  - Softmax is computed un-normalized in [key, query] layout (exp on ScalarE,
    no max subtraction -- scores are O(1) by construction), the denominator
    comes from a ones-row matmul, and the reciprocal is broadcast via a rank-1
    PE matmul.
  - V bias is folded into the reversion bias host-side:
    br_eff = br + bv @ wr (exact because softmax rows sum to 1).
"""

import os
import sys

import numpy as np

# ---- problem constants (hardcoded per contract) ----
B_TOTAL = 16
N_CORES = 8
B = B_TOTAL // N_CORES  # per-core batch
LT, DT = 512, 768       # text tokens / dim
LI, DI = 576, 1024      # image tokens / dim
H, NH, HD = 2048, 8, 256
FF = 128
NEG_SCALE = 1.0 / 16.0  # 1/sqrt(HD)

_BUILD_CACHE: dict = {}


def _ensure_import_path():
    try:
        import concourse  # noqa: F401
    except ModuleNotFoundError:
        for p in ("/opt/trn_rl_repo", "/root/.axon_site/_ro/trn_rl_repo"):
            if os.path.isdir(p) and p not in sys.path:
                sys.path.insert(0, p)


def build_module(use_f32r: bool = True):
    """Build (and cache) the Bass module for one core's work."""
    phases = os.environ.get("KERNEL_PHASES", "12345")
    nb = int(os.environ.get("KERNEL_B", str(B)))
    key = ("nc", use_f32r, phases, nb)
    if key in _BUILD_CACHE:
        return _BUILD_CACHE[key]
    _ensure_import_path()
    from contextlib import ExitStack

    import concourse.bacc as bacc
    import concourse.bass as bass  # noqa: F401
    import concourse.mybir as mybir
    import concourse.tile as tile
    from concourse.masks import make_identity

    f32 = mybir.dt.float32
    f32r = mybir.dt.float32r
    AF = mybir.ActivationFunctionType
    ALU = mybir.AluOpType

    def r(ap):
        return ap.bitcast(f32r) if use_f32r else ap

    nc = bacc.Bacc("TRN2", target_bir_lowering=False, debug=False, num_devices=N_CORES)

    text = nc.dram_tensor("text", [B, LT, DT], f32, kind="ExternalInput").ap()
    image = nc.dram_tensor("image", [B, LI, DI], f32, kind="ExternalInput").ap()
    wq = nc.dram_tensor("wq", [DT, H], f32, kind="ExternalInput").ap()
    wk = nc.dram_tensor("wk", [DI, H], f32, kind="ExternalInput").ap()
    wv = nc.dram_tensor("wv", [DI, H], f32, kind="ExternalInput").ap()
    wr = nc.dram_tensor("wr", [H, DT], f32, kind="ExternalInput").ap()
    w1 = nc.dram_tensor("w1", [DT, FF], f32, kind="ExternalInput").ap()
    w2 = nc.dram_tensor("w2", [FF, DT], f32, kind="ExternalInput").ap()
    bq = nc.dram_tensor("bq", [H], f32, kind="ExternalInput").ap()
    bk = nc.dram_tensor("bk", [H], f32, kind="ExternalInput").ap()
    b1 = nc.dram_tensor("b1", [FF], f32, kind="ExternalInput").ap()
    b2 = nc.dram_tensor("b2", [DT], f32, kind="ExternalInput").ap()
    breff = nc.dram_tensor("breff", [DT], f32, kind="ExternalInput").ap()
    gamma = nc.dram_tensor("gamma", [DT], f32, kind="ExternalInput").ap()
    beta = nc.dram_tensor("beta", [DT], f32, kind="ExternalInput").ap()
    out = nc.dram_tensor("out", [B, LT, DT], f32, kind="ExternalOutput").ap()

    def bcast_row(src, parts, n):
        # DRAM [n] -> SBUF [parts, n] broadcast over partitions
        return bass.AP(tensor=src.tensor, offset=src.offset, ap=[[0, parts], *src.ap])

    with tile.TileContext(nc) as tc, ExitStack() as ctx:
        const = ctx.enter_context(tc.tile_pool(name="const", bufs=1))
        ident = const.tile([128, 128], f32)
        make_identity(nc, ident)
        ones_col = const.tile([128, 1], f32)
        ones_tmp = const.tile([128, 1], f32)
        nc.vector.memset(ones_tmp, 1.0)
        nc.vector.tensor_copy(out=r(ones_col), in_=ones_tmp)
        ones_row = const.tile([1, 128], f32)
        nc.vector.memset(ones_row, 1.0)
        eps_t = const.tile([128, 1], f32)
        nc.vector.memset(eps_t, 1e-5)

        bq_sb = const.tile([128, H // 128], f32)
        nc.sync.dma_start(out=bq_sb, in_=bq.rearrange("(j p) -> p j", p=128))
        bk_sb = const.tile([128, H // 128], f32)
        nc.sync.dma_start(out=bk_sb, in_=bk.rearrange("(j p) -> p j", p=128))
        b1_sb = const.tile([128, 1], f32)
        nc.sync.dma_start(out=b1_sb, in_=b1.rearrange("(j p) -> p j", p=128))
        b2_sb = const.tile([128, DT // 128], f32)
        nc.sync.dma_start(out=b2_sb, in_=b2.rearrange("(j p) -> p j", p=128))
        breff_sb = const.tile([128, DT // 128], f32)
        nc.sync.dma_start(out=breff_sb, in_=breff.rearrange("(j p) -> p j", p=128))
        gam_sb = const.tile([128, DT], f32)
        nc.sync.dma_start(out=gam_sb, in_=bcast_row(gamma, 128, DT))
        bet_sb = const.tile([128, DT], f32)
        nc.sync.dma_start(out=bet_sb, in_=bcast_row(beta, 128, DT))
        w1_sb = const.tile([128, DT // 128, FF], f32)
        nc.sync.dma_start(out=r(w1_sb), in_=r(w1.rearrange("(c p) n -> p c n", p=128)))
        w2_sb = const.tile([128, DT], f32)
        nc.sync.dma_start(out=r(w2_sb), in_=r(w2))

        # global PSUM pools: 4 + 2 + 2 = 8 banks
        psum = ctx.enter_context(tc.tile_pool(name="psum", bufs=4, space="PSUM"))
        psd_pool = ctx.enter_context(tc.tile_pool(name="psd", bufs=2, space="PSUM"))
        pstr_pool = ctx.enter_context(tc.tile_pool(name="pstr", bufs=2, space="PSUM"))

        NT = LT // 128          # 4 text token chunks
        NCD = DT // 128         # 6 text feature chunks
        NCI = DI // 128         # 8 image feature chunks
        ITC = [(t, 128 if t < 4 else LI - 512) for t in range(5)]  # image tok chunks

        for b in range(nb):
            with ExitStack() as bctx:
                actp = bctx.enter_context(tc.tile_pool(name=f"act{b}", bufs=1))

                # ---------- phase 1: load + transpose inputs to feature-major
                text_f = [actp.tile([128, LT], f32, tag=f"textf{c}", name=f"textf{c}") for c in range(NCD)]
                image_f = [actp.tile([128, LI], f32, tag=f"imgf{c}", name=f"imgf{c}") for c in range(NCI)]
                with tc.tile_pool(name=f"stage{b}", bufs=2) as stage:
                    for t in range(NT if "1" in phases else 0):
                        st = stage.tile([128, DT], f32, tag="stg_t", name="stg_t")
                        nc.sync.dma_start(out=st, in_=text[b, t * 128:(t + 1) * 128, :])
                        for c in range(NCD):
                            ps = pstr_pool.tile([128, 128], f32, tag="ptr", name="ptr")
                            nc.tensor.transpose(ps, st[:, c * 128:(c + 1) * 128], ident)
                            nc.vector.tensor_copy(
                                out=r(text_f[c][:, t * 128:(t + 1) * 128]), in_=ps)
                    for t, pt in (ITC if "1" in phases else []):
                        st = stage.tile([128, DI], f32, tag="stg_i", name="stg_i")
                        nc.sync.dma_start(
                            out=st[:pt], in_=image[b, t * 128:t * 128 + pt, :])
                        for c in range(NCI):
                            ps = pstr_pool.tile([128, 128], f32, tag="ptr", name="ptr")
                            nc.tensor.transpose(
                                ps[:, :pt], st[:pt, c * 128:(c + 1) * 128],
                                ident[:pt, :pt])
                            nc.vector.tensor_copy(
                                out=r(image_f[c][:, t * 128:t * 128 + pt]),
                                in_=ps[:, :pt])

                # ---------- phase 2: per-head attention, x accumulated feature-major
                xf = [actp.tile([128, LT], f32, tag=f"xf{c}", name=f"xf{c}") for c in range(H // 128)]
                with tc.tile_pool(name=f"attn{b}", bufs=2) as ap_pool:
                    for h in range(NH if "2" in phases else 0):
                        wq_h = ap_pool.tile([128, NCD, HD], f32, tag="wqh", name="wqh")
                        nc.sync.dma_start(
                            out=r(wq_h),
                            in_=r(wq.rearrange("(c p) n -> p c n", p=128)[
                                :, :, h * HD:(h + 1) * HD]))
                        wk_h = ap_pool.tile([128, NCI, HD], f32, tag="wkh", name="wkh")
                        nc.sync.dma_start(
                            out=r(wk_h),
                            in_=r(wk.rearrange("(c p) n -> p c n", p=128)[
                                :, :, h * HD:(h + 1) * HD]))
                        wv_h = ap_pool.tile([128, NCI, HD], f32, tag="wvh", name="wvh")
                        nc.sync.dma_start(
                            out=r(wv_h),
                            in_=r(wv.rearrange("(c p) n -> p c n", p=128)[
                                :, :, h * HD:(h + 1) * HD]))

                        # Q_h feature-major [256, 512] (+bq)
                        q_h = ap_pool.tile([128, 2, LT], f32, tag="qh", name="qh")
                        for m in range(2):
                            pq = psum.tile([128, LT], f32, tag="ps512", name="ps512")
                            for c in range(NCD):
                                nc.tensor.matmul(
                                    pq, r(wq_h[:, c, m * 128:(m + 1) * 128]),
                                    r(text_f[c]), start=(c == 0), stop=(c == NCD - 1))
                            nc.scalar.activation(
                                out=r(q_h[:, m, :]), in_=pq, func=AF.Identity,
                                bias=bq_sb[:, h * 2 + m:h * 2 + m + 1], scale=1.0)

                        # K_h feature-major [256, 576] (+bk), two 288 halves
                        k_h = ap_pool.tile([128, 2, LI], f32, tag="kh", name="kh")
                        for m in range(2):
                            for n in range(2):
                                pk = psum.tile([128, 288], f32, tag="ps512", name="ps512")
                                for c in range(NCI):
                                    nc.tensor.matmul(
                                        pk, r(wk_h[:, c, m * 128:(m + 1) * 128]),
                                        r(image_f[c][:, n * 288:(n + 1) * 288]),
                                        start=(c == 0), stop=(c == NCI - 1))
                                nc.scalar.activation(
                                    out=r(k_h[:, m, n * 288:(n + 1) * 288]), in_=pk,
                                    func=AF.Identity,
                                    bias=bk_sb[:, h * 2 + m:h * 2 + m + 1], scale=1.0)

                        # V_h token-major [576, 256] (no bias; folded into breff)
                        v_h = ap_pool.tile([128, 5, HD], f32, tag="vh", name="vh")
                        for t, pt in ITC:
                            pv = psum.tile([128, HD], f32, tag="ps512", name="ps512")
                            for c in range(NCI):
                                nc.tensor.matmul(
                                    pv[:pt], r(image_f[c][:, t * 128:t * 128 + pt]),
                                    r(wv_h[:, c, :]), start=(c == 0),
                                    stop=(c == NCI - 1))
                            nc.vector.tensor_copy(out=r(v_h[:pt, t, :]), in_=pv[:pt])

                        # scores^T [key, query] -> exp (unnormalized)
                        e_f = ap_pool.tile([128, 5, LT], f32, tag="ef", name="ef")
                        for t, pt in ITC:
                            ps_s = psum.tile([128, LT], f32, tag="ps512", name="ps512")
                            for m in range(2):
                                nc.tensor.matmul(
                                    ps_s[:pt], r(k_h[:, m, t * 128:t * 128 + pt]),
                                    r(q_h[:, m, :]), start=(m == 0), stop=(m == 1))
                            nc.scalar.activation(
                                out=r(e_f[:pt, t, :]), in_=ps_s[:pt], func=AF.Exp,
                                scale=NEG_SCALE)

                        # denominator row [1, 512] and its reciprocal broadcast
                        pd = psd_pool.tile([1, LT], f32, tag="psd", name="psd")
                        for t, pt in ITC:
                            nc.tensor.matmul(
                                pd, r(ones_col[:pt]), r(e_f[:pt, t, :]),
                                start=(t == 0), stop=(t == 4))
                        recip = ap_pool.tile([1, LT], f32, tag="recip", name="recip")
                        nc.vector.reciprocal(out=recip, in_=pd)
                        pbc = psum.tile([128, LT], f32, tag="ps512", name="ps512")
                        nc.tensor.matmul(pbc, ones_row, recip)
                        bcast = ap_pool.tile([128, LT], f32, tag="bcast", name="bcast")
                        nc.scalar.activation(out=bcast, in_=pbc, func=AF.Copy)

                        # x^T[h] = V^T @ E, normalized by bcast
                        for m in range(2):
                            px = psum.tile([128, LT], f32, tag="ps512", name="ps512")
                            for t, pt in ITC:
                                nc.tensor.matmul(
                                    px, r(v_h[:pt, t, m * 128:(m + 1) * 128]),
                                    r(e_f[:pt, t, :]), start=(t == 0), stop=(t == 4))
                            nc.vector.tensor_mul(
                                out=r(xf[h * 2 + m]), in0=px, in1=bcast)

                # ---------- phase 3: reversion out = x @ wr + br_eff (feature-major)
                out_f = [actp.tile([128, LT], f32, tag=f"outf{m}", name=f"outf{m}") for m in range(NCD)]
                with tc.tile_pool(name=f"wrp{b}", bufs=1) as wrp, \
                        tc.tile_pool(name=f"rev{b}", bufs=2) as rev:
                    wr_sb = wrp.tile([128, H // 128, DT], f32, tag="wr", name="wr")
                    for cc in range(4 if "3" in phases else 0):
                        nc.sync.dma_start(
                            out=r(wr_sb[:, cc * 4:(cc + 1) * 4, :]),
                            in_=r(wr.rearrange("(c p) n -> p c n", p=128)[
                                :, cc * 4:(cc + 1) * 4, :]))
                    for m in range(NCD if "3" in phases else 0):
                        po = psum.tile([128, LT], f32, tag="ps512", name="ps512")
                        for c in range(H // 128):
                            nc.tensor.matmul(
                                po, r(wr_sb[:, c, m * 128:(m + 1) * 128]), r(xf[c]),
                                start=(c == 0), stop=(c == H // 128 - 1))
                        nc.scalar.activation(
                            out=r(out_f[m]), in_=po, func=AF.Identity,
                            bias=breff_sb[:, m:m + 1], scale=1.0)

                    # ---------- phase 4: FF + residual (in place on out_f)
                    ph = psum.tile([128, LT], f32, tag="ps512", name="ps512")
                    for c in range(NCD if "4" in phases else 0):
                        nc.tensor.matmul(
                            ph, r(w1_sb[:, c, :]), r(out_f[c]),
                            start=(c == 0), stop=(c == NCD - 1))
                    h_sb = rev.tile([128, LT], f32, tag="hsb", name="hsb")
                    if "4" in phases:
                        nc.scalar.activation(
                            out=r(h_sb), in_=ph, func=AF.Relu, bias=b1_sb,
                            scale=1.0)
                    for m in range(NCD if "4" in phases else 0):
                        pf = psum.tile([128, LT], f32, tag="ps512", name="ps512")
                        nc.tensor.matmul(
                            pf, r(w2_sb[:, m * 128:(m + 1) * 128]), r(h_sb))
                        ff_sb = rev.tile([128, LT], f32, tag="ffsb", name="ffsb")
                        nc.scalar.activation(
                            out=ff_sb, in_=pf, func=AF.Identity,
                            bias=b2_sb[:, m:m + 1], scale=1.0)
                        nc.vector.tensor_add(out=r(out_f[m]), in0=out_f[m], in1=ff_sb)

                    # ---------- phase 5: transpose back + LayerNorm + store
                    for t in range(NT if "5" in phases else 0):
                        res_t = rev.tile([128, DT], f32, tag="rest", name="rest")
                        for c in range(NCD):
                            ps = pstr_pool.tile([128, 128], f32, tag="ptr", name="ptr")
                            nc.tensor.transpose(
                                ps, out_f[c][:, t * 128:(t + 1) * 128], ident)
                            nc.vector.tensor_copy(
                                out=res_t[:, c * 128:(c + 1) * 128], in_=ps)
                        stats = rev.tile([128, 3, 6], f32, tag="stats", name="stats")
                        for s in range(3):
                            nc.vector.bn_stats(
                                out=stats[:, s, :], in_=res_t[:, s * 256:(s + 1) * 256])
                        mv = rev.tile([128, 2], f32, tag="mv", name="mv")
                        nc.vector.bn_aggr(out=mv, in_=stats)
                        std = rev.tile([128, 1], f32, tag="std", name="std")
                        nc.scalar.activation(
                            out=std, in_=mv[:, 1:2], func=AF.Sqrt, bias=eps_t,
                            scale=1.0)
                        rstd = rev.tile([128, 1], f32, tag="rstd", name="rstd")
                        nc.vector.reciprocal(out=rstd, in_=std)
                        y = rev.tile([128, DT], f32, tag="y", name="y")
                        nc.vector.tensor_scalar(
                            out=y, in0=res_t, scalar1=mv[:, 0:1], scalar2=rstd,
                            op0=ALU.subtract, op1=ALU.mult)
                        nc.vector.tensor_mul(out=y, in0=y, in1=gam_sb)
                        nc.vector.tensor_add(out=y, in0=y, in1=bet_sb)
                        nc.sync.dma_start(
                            out=out[b, t * 128:(t + 1) * 128, :], in_=y)

    nc.compile()
    _BUILD_CACHE[key] = nc
    return nc


def _prep_in_maps(inputs):
    def f32c(x):
        return np.ascontiguousarray(np.asarray(x, dtype=np.float32))

    text = f32c(inputs["text"])
    image = f32c(inputs["image"])
    wr = np.asarray(inputs["wr"], dtype=np.float64)
    bv = np.asarray(inputs["bv"], dtype=np.float64)
    br = np.asarray(inputs["br"], dtype=np.float64)
    breff = (br + bv @ wr).astype(np.float32)

    shared = {
        "wq": f32c(inputs["wq"]), "wk": f32c(inputs["wk"]),
        "wv": f32c(inputs["wv"]), "wr": f32c(inputs["wr"]),
        "w1": f32c(inputs["w1"]), "w2": f32c(inputs["w2"]),
        "bq": f32c(inputs["bq"]), "bk": f32c(inputs["bk"]),
        "b1": f32c(inputs["b1"]), "b2": f32c(inputs["b2"]),
        "breff": breff, "gamma": f32c(inputs["gamma"]),
        "beta": f32c(inputs["beta"]),
    }
    in_maps = []
    for c in range(N_CORES):
        m = dict(shared)
        m["text"] = text[c * B:(c + 1) * B]
        m["image"] = image[c * B:(c + 1) * B]
        in_maps.append(m)
    return in_maps


def kernel(**inputs) -> np.ndarray:
    _ensure_import_path()
    from concourse.bass_utils import run_bass_kernel_spmd

    nc = build_module()
    in_maps = _prep_in_maps(inputs)
    res = run_bass_kernel_spmd(nc, in_maps, core_ids=list(range(N_CORES)))
    return np.concatenate([res.results[c]["out"] for c in range(N_CORES)], axis=0)



# revision 12
# speedup vs baseline: 1.3772x; 1.3772x over previous
"""Trainium2 Bass kernel for nn_AttentionLayer (cross-attention + FF + LayerNorm).

V2 strategy (data-parallel over batch, 2 per core):
  - bf16 on-chip activations/weights for projections + attention (full PE
    rate at any moving dim, 2x DVE, half DMA); f32r for the reversion
    (wr, xf) and the residual/LN path so the dominant error terms stay f32.
  - Inputs loaded FEATURE-major directly via DMA-transpose (XBAR), removing
    all phase-1 PE transposes and PSUM->SBUF copies.
  - wr loaded once into a const pool (not per batch), prefetched during
    attention of batch 0.
  - Phase 2 software-pipelined: the softmax tail of head i-1 (denominator,
    reciprocal, broadcast, V^T@E) is emitted inside head i's projection
    matmuls, so PE never stalls on Act/DVE round trips (keeps the PE
    p-state at full clock).
  - Softmax un-normalized in [key, query] layout; denominator via ones-row
    matmul; normalization folded into the PSUM->SBUF move of x (TT mult
    with the PE-broadcast reciprocal, both operands in PSUM).
  - LayerNorm: stats computed feature-major with ones-column matmuls
    (sum, sum-of-squares), rstd = exp(-0.5*ln(var+eps)) so every Act
    function lives in one act-table set (no table reloads); the normalize
    is fused into the PSUM->SBUF copies after the transpose back to
    token-major (per-partition scale/bias).
  - V bias folded into the reversion bias host-side: breff = br + bv @ wr.
"""

import os
import sys

import numpy as np

# ---- problem constants (hardcoded per contract) ----
B_TOTAL = 16
N_CORES = 8
B = B_TOTAL // N_CORES  # per-core batch
LT, DT = 512, 768       # text tokens / dim
LI, DI = 576, 1024      # image tokens / dim
H, NH, HD = 2048, 8, 256
FF = 128
ISCALE = 1.0 / 16.0     # 1/sqrt(HD)
NPAIR = B * NH          # 16 (batch, head) pairs per core
ITC = [(t, 128 if t < 4 else LI - 512) for t in range(5)]  # image tok chunks
NCD = DT // 128         # 6
NCI = DI // 128         # 8
NT = LT // 128          # 4

_BUILD_CACHE: dict = {}


def _ensure_import_path():
    try:
        import concourse  # noqa: F401
    except ModuleNotFoundError:
        for p in ("/opt/trn_rl_repo", "/root/.axon_site/_ro/trn_rl_repo"):
            if os.path.isdir(p) and p not in sys.path:
                sys.path.insert(0, p)


def build_module(apply_gamma: bool = False):
    key = ("v2", apply_gamma)
    if key in _BUILD_CACHE:
        return _BUILD_CACHE[key]
    _ensure_import_path()
    from contextlib import ExitStack

    import concourse.bacc as bacc
    import concourse.bass as bass  # noqa: F401
    import concourse.mybir as mybir
    import concourse.tile as tile
    from concourse.masks import make_identity

    f32 = mybir.dt.float32
    f32r = mybir.dt.float32r
    bf16 = mybir.dt.bfloat16
    AF = mybir.ActivationFunctionType
    ALU = mybir.AluOpType

    def r(ap):
        return ap.bitcast(f32r)

    nc = bacc.Bacc("TRN2", target_bir_lowering=False, debug=False, num_devices=N_CORES)

    text = nc.dram_tensor("text", [B, LT, DT], bf16, kind="ExternalInput").ap()
    image = nc.dram_tensor("image", [B, LI, DI], bf16, kind="ExternalInput").ap()
    wq = nc.dram_tensor("wq", [DT, H], bf16, kind="ExternalInput").ap()
    wk = nc.dram_tensor("wk", [DI, H], bf16, kind="ExternalInput").ap()
    wv = nc.dram_tensor("wv", [DI, H], bf16, kind="ExternalInput").ap()
    wr = nc.dram_tensor("wr", [H, DT], f32, kind="ExternalInput").ap()
    w1 = nc.dram_tensor("w1", [DT, FF], bf16, kind="ExternalInput").ap()
    w2 = nc.dram_tensor("w2", [FF, DT], bf16, kind="ExternalInput").ap()
    bq = nc.dram_tensor("bq", [H], f32, kind="ExternalInput").ap()
    bk = nc.dram_tensor("bk", [H], f32, kind="ExternalInput").ap()
    b1 = nc.dram_tensor("b1", [FF], f32, kind="ExternalInput").ap()
    b2 = nc.dram_tensor("b2", [DT], f32, kind="ExternalInput").ap()
    breff = nc.dram_tensor("breff", [DT], f32, kind="ExternalInput").ap()
    gamma = nc.dram_tensor("gamma", [DT], f32, kind="ExternalInput").ap()
    beta = nc.dram_tensor("beta", [DT], f32, kind="ExternalInput").ap()
    out = nc.dram_tensor("out", [B, LT, DT], f32, kind="ExternalOutput").ap()

    def bcast_row(src, parts, n):
        return bass.AP(tensor=src.tensor, offset=src.offset, ap=[[0, parts], *src.ap])

    with tile.TileContext(nc) as tc, ExitStack() as ctx:
        const = ctx.enter_context(tc.tile_pool(name="const", bufs=1))
        ident = const.tile([128, 128], f32)
        make_identity(nc, ident)
        ones_col_b = const.tile([128, 1], bf16)
        nc.vector.memset(ones_col_b, 1.0)
        ones_row_b = const.tile([1, 128], bf16)
        nc.vector.memset(ones_row_b, 1.0)
        ones_col_f = const.tile([128, 1], f32)
        nc.vector.memset(ones_col_f, 1.0)
        eps_t = const.tile([128, 1], f32)
        nc.vector.memset(eps_t, 1e-5)

        bq_sb = const.tile([128, H // 128], f32)
        nc.sync.dma_start(out=bq_sb, in_=bq.rearrange("(j p) -> p j", p=128))
        bk_sb = const.tile([128, H // 128], f32)
        nc.sync.dma_start(out=bk_sb, in_=bk.rearrange("(j p) -> p j", p=128))
        b1_sb = const.tile([128, 1], f32)
        nc.sync.dma_start(out=b1_sb, in_=b1.rearrange("(j p) -> p j", p=128))
        b2_sb = const.tile([128, NCD], f32)
        nc.sync.dma_start(out=b2_sb, in_=b2.rearrange("(j p) -> p j", p=128))
        breff_sb = const.tile([128, NCD], f32)
        nc.sync.dma_start(out=breff_sb, in_=breff.rearrange("(j p) -> p j", p=128))
        w1_sb = const.tile([128, NCD, FF], bf16)
        nc.sync.dma_start(out=w1_sb, in_=w1.rearrange("(c p) n -> p c n", p=128))
        w2_sb = const.tile([128, DT], bf16)
        nc.sync.dma_start(out=w2_sb, in_=w2)
        wr_sb = const.tile([128, H // 128, DT], f32)  # loaded in 4 chunks mid-flight
        if apply_gamma:
            gam_sb = const.tile([128, DT], f32)
            nc.sync.dma_start(out=gam_sb, in_=bcast_row(gamma, 128, DT))
            bet_sb = const.tile([128, DT], f32)
            nc.sync.dma_start(out=bet_sb, in_=bcast_row(beta, 128, DT))

        psum = ctx.enter_context(tc.tile_pool(name="psum", bufs=4, space="PSUM"))
        psd = ctx.enter_context(tc.tile_pool(name="psd", bufs=2, space="PSUM"))
        pstr = ctx.enter_context(tc.tile_pool(name="pstr", bufs=2, space="PSUM"))
        tfp = ctx.enter_context(tc.tile_pool(name="tfp", bufs=2))
        hwp = ctx.enter_context(tc.tile_pool(name="hwp", bufs=2))
        atp = ctx.enter_context(tc.tile_pool(name="atp", bufs=2))
        xfp = ctx.enter_context(tc.tile_pool(name="xfp", bufs=1))
        ofp = ctx.enter_context(tc.tile_pool(name="ofp", bufs=1))
        p5p = ctx.enter_context(tc.tile_pool(name="p5p", bufs=2))

        TFI: dict = {}
        W: dict = {}
        S: dict = {}
        XF: dict = {}
        OF: dict = {}
        SQ: dict = {}

        def load_inputs(b):
            tfs = [tfp.tile([128, LT], bf16, tag=f"textf{c}", name=f"textf{c}")
                   for c in range(NCD)]
            ifs = [tfp.tile([128, LI], bf16, tag=f"imgf{c}", name=f"imgf{c}")
                   for c in range(NCI)]
            for c in range(NCD):
                nc.sync.dma_start_transpose(
                    out=tfs[c], in_=text[b, :, c * 128:(c + 1) * 128])
            for c in range(NCI):
                nc.sync.dma_start_transpose(
                    out=ifs[c], in_=image[b, :, c * 128:(c + 1) * 128])
            TFI[b] = (tfs, ifs)

        def load_weights(i):
            h = i % NH
            wq_h = hwp.tile([128, NCD, HD], bf16, tag="wqh", name="wqh")
            nc.sync.dma_start(
                out=wq_h,
                in_=wq.rearrange("(c p) n -> p c n", p=128)[:, :, h * HD:(h + 1) * HD])
            wk_h = hwp.tile([128, NCI, HD], bf16, tag="wkh", name="wkh")
            nc.sync.dma_start(
                out=wk_h,
                in_=wk.rearrange("(c p) n -> p c n", p=128)[:, :, h * HD:(h + 1) * HD])
            wv_h = hwp.tile([128, NCI, HD], bf16, tag="wvh", name="wvh")
            nc.sync.dma_start(
                out=wv_h,
                in_=wv.rearrange("(c p) n -> p c n", p=128)[:, :, h * HD:(h + 1) * HD])
            W[i] = (wq_h, wk_h, wv_h)

        def load_wr_chunk(cc):
            nc.sync.dma_start(
                out=r(wr_sb[:, cc * 4:(cc + 1) * 4, :]),
                in_=r(wr.rearrange("(c p) n -> p c n", p=128)[:, cc * 4:(cc + 1) * 4, :]))

        def alloc_xf(b):
            XF[b] = [xfp.tile([128, LT], f32, tag=f"xf{c}", name=f"xf{c}")
                     for c in range(H // 128)]

        def emit_Q(i):
            b, h = divmod(i, NH)
            tfs, _ = TFI[b]
            wq_h, _, _ = W[i]
            q_h = atp.tile([128, 2, LT], bf16, tag="qh", name="qh")
            for m in range(2):
                pq = psum.tile([128, LT], f32, tag="ps512", name="ps512")
                for c in range(NCD):
                    nc.tensor.matmul(pq, wq_h[:, c, m * 128:(m + 1) * 128], tfs[c],
                                     start=(c == 0), stop=(c == NCD - 1))
                nc.scalar.activation(
                    out=q_h[:, m, :], in_=pq, func=AF.Identity,
                    bias=bq_sb[:, h * 2 + m:h * 2 + m + 1], scale=1.0)
            S[i] = {"q": q_h}

        def emit_K(i):
            b, h = divmod(i, NH)
            _, ifs = TFI[b]
            _, wk_h, _ = W[i]
            k_h = atp.tile([128, 2, LI], bf16, tag="kh", name="kh")
            for m in range(2):
                for n in range(2):
                    pk = psum.tile([128, 288], f32, tag="ps512", name="ps512")
                    for c in range(NCI):
                        nc.tensor.matmul(
                            pk, wk_h[:, c, m * 128:(m + 1) * 128],
                            ifs[c][:, n * 288:(n + 1) * 288],
                            start=(c == 0), stop=(c == NCI - 1))
                    nc.scalar.activation(
                        out=k_h[:, m, n * 288:(n + 1) * 288], in_=pk,
                        func=AF.Identity,
                        bias=bk_sb[:, h * 2 + m:h * 2 + m + 1], scale=1.0)
            S[i]["k"] = k_h

        def emit_V(i):
            b, h = divmod(i, NH)
            _, ifs = TFI[b]
            _, _, wv_h = W[i]
            v_h = atp.tile([128, 5, HD], bf16, tag="vh", name="vh")
            for t, pt in ITC:
                pv = psum.tile([128, HD], f32, tag="ps512", name="ps512")
                for c in range(NCI):
                    nc.tensor.matmul(
                        pv[:pt], ifs[c][:, t * 128:t * 128 + pt], wv_h[:, c, :],
                        start=(c == 0), stop=(c == NCI - 1))
                nc.vector.tensor_copy(out=v_h[:pt, t, :], in_=pv[:pt])
            S[i]["v"] = v_h

        def emit_scores(i):
            q_h, k_h = S[i]["q"], S[i]["k"]
            e_f = atp.tile([128, 5, LT], bf16, tag="ef", name="ef")
            for t, pt in ITC:
                ps_s = psum.tile([128, LT], f32, tag="ps512", name="ps512")
                for m in range(2):
                    nc.tensor.matmul(
                        ps_s[:pt], k_h[:, m, t * 128:t * 128 + pt], q_h[:, m, :],
                        start=(m == 0), stop=(m == 1))
                nc.scalar.activation(
                    out=e_f[:pt, t, :], in_=ps_s[:pt], func=AF.Exp, scale=ISCALE)
            S[i]["e"] = e_f

        def emit_pd(i):
            e_f = S[i]["e"]
            pd2 = psd.tile([33, LT], f32, tag="psd", name="psd")
            for t, pt in ITC:
                nc.tensor.matmul(pd2[0:1, :], ones_col_b[:pt], e_f[:pt, t, :],
                                 start=(t == 0), stop=(t == 4))
            recip = atp.tile([1, LT], bf16, tag="recip", name="recip")
            with nc.allow_low_precision(reason="softmax recip feeds bf16 matmul"):
                nc.vector.reciprocal(out=recip, in_=pd2[0:1, :])
            S[i]["recip"] = recip

        def emit_pbc(i):
            pbc = psum.tile([128, LT], f32, tag="ps512", name="ps512")
            nc.tensor.matmul(pbc, ones_row_b, S[i]["recip"])
            S[i]["pbc"] = pbc

        def emit_px(i):
            b, h = divmod(i, NH)
            v_h, e_f, pbc = S[i]["v"], S[i]["e"], S[i]["pbc"]
            for m in range(2):
                px = psum.tile([128, LT], f32, tag="ps512", name="ps512")
                for t, pt in ITC:
                    nc.tensor.matmul(
                        px, v_h[:pt, t, m * 128:(m + 1) * 128], e_f[:pt, t, :],
                        start=(t == 0), stop=(t == 4))
                nc.vector.tensor_mul(out=XF[b][h * 2 + m], in0=px, in1=pbc)
            S[i] = None  # release references

        def emit_rev(b):
            ofs = [ofp.tile([128, LT], f32, tag=f"of{m}", name=f"of{m}")
                   for m in range(NCD)]
            ofb = [ofp.tile([128, LT], bf16, tag=f"ofb{m}", name=f"ofb{m}")
                   for m in range(NCD)]
            for m in range(NCD):
                po = psum.tile([128, LT], f32, tag="ps512", name="ps512")
                for c in range(H // 128):
                    nc.tensor.matmul(po, r(wr_sb[:, c, m * 128:(m + 1) * 128]),
                                     r(XF[b][c]),
                                     start=(c == 0), stop=(c == H // 128 - 1))
                nc.scalar.activation(
                    out=ofs[m], in_=po, func=AF.Identity,
                    bias=breff_sb[:, m:m + 1], scale=1.0)
                nc.vector.tensor_scalar(
                    out=ofb[m], in0=po, scalar1=breff_sb[:, m:m + 1], scalar2=None,
                    op0=ALU.add)
            OF[b] = (ofs, ofb)

        def emit_ff(b):
            ofs, ofb = OF[b]
            ph = psum.tile([128, LT], f32, tag="ps512", name="ps512")
            for c in range(NCD):
                nc.tensor.matmul(ph, w1_sb[:, c, :], ofb[c],
                                 start=(c == 0), stop=(c == NCD - 1))
            h_sb = p5p.tile([128, LT], bf16, tag="hsb", name="hsb")
            nc.scalar.activation(out=h_sb, in_=ph, func=AF.Relu, bias=b1_sb,
                                 scale=1.0)
            for m in range(NCD):
                pf = psum.tile([128, LT], f32, tag="ps512", name="ps512")
                nc.tensor.matmul(pf, w2_sb[:, m * 128:(m + 1) * 128], h_sb)
                ff_sb = p5p.tile([128, LT], f32, tag="ffsb", name="ffsb")
                nc.scalar.activation(
                    out=ff_sb, in_=pf, func=AF.Identity,
                    bias=b2_sb[:, m:m + 1], scale=1.0)
                nc.vector.tensor_add(out=ofs[m], in0=ofs[m], in1=ff_sb)
                # square for the variance sum, emitted early so Act keeps pace
                sq = ofp.tile([128, LT], bf16, tag=f"sq{m}", name=f"sq{m}")
                nc.scalar.activation(out=sq, in_=ofs[m], func=AF.Square, scale=1.0)
                SQ.setdefault(b, []).append(sq)

        def emit_stats(b):
            ofs, _ = OF[b]
            pstat = psd.tile([33, LT], f32, tag="psd", name="psd")
            for m in range(NCD):
                nc.tensor.matmul(pstat[0:1, :], r(ones_col_f), r(ofs[m]),
                                 start=(m == 0), stop=(m == NCD - 1))
            for m in range(NCD):
                nc.tensor.matmul(pstat[32:33, :], ones_col_b, SQ[b][m],
                                 start=(m == 0), stop=(m == NCD - 1))
            srowA = p5p.tile([1, LT], f32, tag="srowA", name="srowA")
            srowB = p5p.tile([1, LT], f32, tag="srowB", name="srowB")
            nc.vector.tensor_copy(out=srowA, in_=pstat[0:1, :])
            nc.vector.tensor_copy(out=srowB, in_=pstat[32:33, :])
            stats = []
            for t in range(NT):
                ptc = pstr.tile([128, 128], f32, tag="ptr", name="ptr")
                nc.tensor.transpose(r(ptc[:, 0:1]),
                                    r(srowA[:, t * 128:(t + 1) * 128]),
                                    r(ident[:1, :1]))
                nc.tensor.transpose(r(ptc[:, 1:2]),
                                    r(srowB[:, t * 128:(t + 1) * 128]),
                                    r(ident[:1, :1]))
                mu_t = p5p.tile([128, 1], f32, tag=f"mu{t}", name=f"mu{t}")
                nc.vector.tensor_scalar(
                    out=mu_t, in0=ptc[:, 0:1], scalar1=1.0 / DT, scalar2=None,
                    op0=ALU.mult)
                musq = p5p.tile([128, 1], f32, tag=f"musq{t}", name=f"musq{t}")
                nc.vector.tensor_mul(out=musq, in0=mu_t, in1=mu_t)
                var_t = p5p.tile([128, 1], f32, tag=f"var{t}", name=f"var{t}")
                nc.vector.scalar_tensor_tensor(
                    out=var_t, in0=ptc[:, 1:2], scalar=1.0 / DT, in1=musq,
                    op0=ALU.mult, op1=ALU.subtract)
                lnv = p5p.tile([128, 1], f32, tag=f"lnv{t}", name=f"lnv{t}")
                nc.scalar.activation(out=lnv, in_=var_t, func=AF.Ln, bias=eps_t,
                                     scale=1.0)
                rstd = p5p.tile([128, 1], f32, tag=f"rstd{t}", name=f"rstd{t}")
                nc.scalar.activation(out=rstd, in_=lnv, func=AF.Exp, scale=-0.5)
                nmr = p5p.tile([128, 1], f32, tag=f"nmr{t}", name=f"nmr{t}")
                nc.vector.tensor_scalar(
                    out=nmr, in0=mu_t, scalar1=rstd, scalar2=-1.0,
                    op0=ALU.mult, op1=ALU.mult)
                stats.append((rstd, nmr))
            OF[b] = (ofs, stats)

        def emit_ln_chunk(b, t):
            ofs, stats = OF[b]
            rstd, nmr = stats[t]
            y = p5p.tile([128, DT], f32, tag="y", name="y")
            for c in range(NCD):
                ptr_ = pstr.tile([128, 128], f32, tag="ptr", name="ptr")
                nc.tensor.transpose(r(ptr_), r(ofs[c][:, t * 128:(t + 1) * 128]),
                                    r(ident))
                dst = y[:, c * 128:(c + 1) * 128]
                if c % 2 == 0:
                    nc.scalar.activation(out=dst, in_=ptr_, func=AF.Identity,
                                         bias=nmr, scale=rstd)
                else:
                    nc.vector.tensor_scalar(
                        out=dst, in0=ptr_, scalar1=rstd, scalar2=nmr,
                        op0=ALU.mult, op1=ALU.add)
            if apply_gamma:
                nc.vector.tensor_mul(out=y, in0=y, in1=gam_sb)
                nc.vector.tensor_add(out=y, in0=y, in1=bet_sb)
            nc.sync.dma_start(out=out[b, t * 128:(t + 1) * 128, :], in_=y)

        # ---------------- emission schedule ----------------
        load_inputs(0)
        load_weights(0)
        alloc_xf(0)
        prev = None
        for i in range(NPAIR):
            if i + 1 < NPAIR:
                load_weights(i + 1)
            if i == 6:
                load_inputs(1)
            if 3 <= i <= 6:
                load_wr_chunk(i - 3)
            emit_Q(i)
            if prev is not None:
                emit_pd(prev)
            emit_K(i)
            if prev is not None:
                emit_pbc(prev)
                emit_px(prev)
            emit_V(i)
            emit_scores(i)
            if i == 8:
                emit_rev(0)
                emit_ff(0)
                alloc_xf(1)
            elif i == 9:
                emit_stats(0)
            elif 10 <= i <= 13:
                emit_ln_chunk(0, i - 10)
            prev = i
        emit_pd(prev)
        emit_pbc(prev)
        emit_px(prev)
        emit_rev(1)
        emit_ff(1)
        emit_stats(1)
        for t in range(NT):
            emit_ln_chunk(1, t)

    nc.compile()
    _BUILD_CACHE[key] = nc
    return nc


def _prep_in_maps(inputs):
    import ml_dtypes

    def bf(x):
        return np.ascontiguousarray(np.asarray(x, dtype=np.float32).astype(
            ml_dtypes.bfloat16))

    def f32c(x):
        return np.ascontiguousarray(np.asarray(x, dtype=np.float32))

    text = bf(inputs["text"])
    image = bf(inputs["image"])
    wr = np.asarray(inputs["wr"], dtype=np.float64)
    bv = np.asarray(inputs["bv"], dtype=np.float64)
    br = np.asarray(inputs["br"], dtype=np.float64)
    breff = (br + bv @ wr).astype(np.float32)

    shared = {
        "wq": bf(inputs["wq"]), "wk": bf(inputs["wk"]), "wv": bf(inputs["wv"]),
        "wr": f32c(inputs["wr"]),
        "w1": bf(inputs["w1"]), "w2": bf(inputs["w2"]),
        "bq": f32c(inputs["bq"]), "bk": f32c(inputs["bk"]),
        "b1": f32c(inputs["b1"]), "b2": f32c(inputs["b2"]),
        "breff": breff, "gamma": f32c(inputs["gamma"]),
        "beta": f32c(inputs["beta"]),
    }
    in_maps = []
    for c in range(N_CORES):
        m = dict(shared)
        m["text"] = text[c * B:(c + 1) * B]
        m["image"] = image[c * B:(c + 1) * B]
        in_maps.append(m)
    return in_maps


def _needs_gamma(inputs):
    g = np.asarray(inputs["gamma"], dtype=np.float32)
    b = np.asarray(inputs["beta"], dtype=np.float32)
    return not (np.all(g == 1.0) and np.all(b == 0.0))


def kernel(**inputs) -> np.ndarray:
    _ensure_import_path()
    from concourse.bass_utils import run_bass_kernel_spmd

    nc = build_module(apply_gamma=_needs_gamma(inputs))
    in_maps = _prep_in_maps(inputs)
    res = run_bass_kernel_spmd(nc, in_maps, core_ids=list(range(N_CORES)))
    return np.concatenate([res.results[c]["out"] for c in range(N_CORES)], axis=0)
